# revision 12
# baseline (speedup 1.0000x reference)
"""Trainium2 Bass kernel for nn_EquivariantTransformerBlock.

Strategy (8 NeuronCores, no collectives, no indirect DMA):
  - Host assigns each node to one of 320 "buckets" of 128 nodes (degree-
    balanced snake packing).  Core c owns buckets [40c, 40c+40); every edge
    goes to the core owning its dst bucket, so all segment sums are local.
  - Host computes the (tiny) equivariant LayerNorm and gathers f[src] so
    fU arrives as a dense per-edge input; all heavy compute (edge MLP,
    per-edge tensor contractions, attention, projection) runs on device.
  - Per-edge compute uses an edges-on-partitions layout (128 edges/chunk):
      * edge MLP on the TensorE in float32r (full PE rate, fp32-like
        precision),
      * the per-edge bilinear contractions (tmp = fU@basis, conv = rw@tmp)
        as broadcast-view products + halving-tree sums on the VectorE in
        FP16 (2x_1p DVE mode; fp16 keeps score error ~5e-4 relative, vs
        bf16 whose ~0.4% error is exp-amplified out of tolerance),
      * per-edge softmax scalars (LeakyReLU, exp, broadcasts, casts) on
        the ScalarE,
      * segment sums as one-hot matmuls (host-precomputed bf16 one-hot)
        accumulated in PSUM per bucket.
  - Softmax without per-node max: two exp variants per edge - A: clamped
    exp(s) (valid while den_A < 1e33), B: exp(s - 140) (valid for hot
    nodes) - and a per-node select after the segment sums.  Softmax is
    shift-invariant, so either variant matches the reference numerically.
"""

import math
from contextlib import ExitStack
from dataclasses import dataclass

import numpy as np

N_NODES = 40000
N_EDGES = 320000
M1, D1 = 8, 4
M2, D2 = 8, 4
LN_EPS = 1e-5
EQ_EPS = 1e-8
IX1 = np.array([0, 1, 1, 1])
IX2 = np.array([0, 1, 1, 1])

N_CORES = 8
BUCKET_N = 128
NB = 40
SCALE = 32.0 ** -0.5
SHIFT_B = 140.0      # pass-B exponent shift
CLAMP_A = 1e34       # pass-A exp clamp
SEL_TH = 1e33        # use pass B when den_A >= SEL_TH


@dataclass
class Cfg:
    nb: int
    kb: int
    b2zero: bool = False

    @property
    def ch(self):
        return self.nb * self.kb

    @property
    def e_pad(self):
        return self.ch * 128


# ---------------------------------------------------------------------------
# Patches: this walrus build allows at most ONE sync wait per instruction.
# ---------------------------------------------------------------------------
_PATCHED = False


def _apply_patches():
    global _PATCHED
    if _PATCHED:
        return
    _PATCHED = True
    import re as _re

    import orjson as _orjson

    import concourse.bass as _bass
    from concourse.tile import TileContext as _TC
    from concourse.vector_clock import ScopedClock as _SC, VectorClock as _VC

    def _drain_and_barrier(self, tick_clock, wait_clock):
        nc = self.nc
        gvals = [int(x) for x in _re.findall(r"\d+", repr(tick_clock.global_clock))]
        nz = [(p, v) for p, v in enumerate(gvals) if v > 0]
        if not nz:
            nc.sync.drain()
        for p, v in nz:
            pvc = _VC()
            pvc.require_at_least(p, v)
            d = nc.sync.drain()
            wait_clock.add_sem_waits(d.ins, _SC({None: pvc}))
        nc.all_engine_barrier()
        assert self.sems is not None
        popped = nc._tile_sem_poison_stack.pop()
        assert popped is self._sem_poison
        nc.clear_and_free_semaphores(list(self.sems.allocated().values()))
        nc.all_engine_barrier()

    def _split_multi_waits(data: bytes) -> bytes:
        j = _orjson.loads(data)
        for fn in j.get("functions", []):
            for bb in fn.get("blocks", []):
                out = []
                for ins in bb.get("instructions", []):
                    si = ins.get("sync_info")
                    ow = (si or {}).get("on_wait") or []
                    if len(ow) > 1:
                        for k, w in enumerate(ow[:-1]):
                            out.append({
                                "debug": ins.get("debug", 0),
                                "engine": ins["engine"],
                                "ins": [],
                                "name": f"{ins['name']}-spw{k}",
                                "opcode": "EventSemaphore",
                                "outs": [],
                                "sync_info": {"on_update": [], "on_wait": [w]},
                            })
                        si["on_wait"] = [ow[-1]]
                    out.append(ins)
                bb["instructions"] = out
        return _orjson.dumps(j)

    _orig_to_json_bytes = _bass.Bass.to_json_bytes

    def _to_json_bytes(self):
        return _split_multi_waits(_orig_to_json_bytes(self))

    _TC._drain_and_barrier = _drain_and_barrier
    _bass.Bass.to_json_bytes = _to_json_bytes


# ---------------------------------------------------------------------------
# Device kernel
# ---------------------------------------------------------------------------
def build_kernel(nc, cfg: Cfg):
    import concourse.bass as bass
    import concourse.mybir as mybir
    from concourse.tile import TileContext

    f32 = mybir.dt.float32
    f32r = mybir.dt.float32r
    fp16 = mybir.dt.float16
    bf16 = mybir.dt.bfloat16
    Alu = mybir.AluOpType
    Act = mybir.ActivationFunctionType

    NBk, KB, CH, E_PAD = cfg.nb, cfg.kb, cfg.ch, cfg.e_pad

    basis_d = nc.dram_tensor("basis_s", (NBk, 128, KB * 64), fp16,
                             kind="ExternalInput")
    fu_d = nc.dram_tensor("fu_s", (NBk, 128, KB * 32), fp16,
                          kind="ExternalInput")
    eft_d = nc.dram_tensor("eft_s", (32, E_PAD), f32r, kind="ExternalInput")
    oh_d = nc.dram_tensor("oh_s", (NBk, 128, KB * 128), bf16,
                          kind="ExternalInput")
    w1t_d = nc.dram_tensor("w1t_s", (32, 64), f32r, kind="ExternalInput")
    b1_d = nc.dram_tensor("b1_s", (64, 1), f32, kind="ExternalInput")
    w2b_d = nc.dram_tensor("w2b_s", (64, 768), f32r, kind="ExternalInput")
    b2_d = nc.dram_tensor("b2r_s", (1, 768), f32r, kind="ExternalInput")
    ones_d = nc.dram_tensor("ones_s", (1, 128), f32r, kind="ExternalInput")
    proj_d = nc.dram_tensor("proj_s", (128, 256), f32, kind="ExternalInput")
    out_d = nc.dram_tensor("out_s", (NBk * 128, 32), f32,
                           kind="ExternalOutput")

    def vap(base, offset, dims):
        return bass.AP(base.tensor, base.offset + offset, dims)

    # MLP1 group sizes (chunks per hps tile; 512 cols keeps f32r at full rate)
    GRPS = []
    i0 = 0
    while i0 < KB:
        ng = min(4, KB - i0)
        GRPS.append((i0, ng))
        i0 += ng

    with TileContext(nc) as tc:
        with ExitStack() as ctx:
            cpool = ctx.enter_context(tc.tile_pool(name="consts", bufs=1))
            w1t_t = cpool.tile([32, 64], f32r)
            nc.sync.dma_start(out=w1t_t[:], in_=w1t_d.ap())
            b1_t = cpool.tile([64, 1], f32)
            nc.sync.dma_start(out=b1_t[:], in_=b1_d.ap())
            w2b_t = cpool.tile([64, 768], f32r)
            nc.sync.dma_start(out=w2b_t[:], in_=w2b_d.ap())
            b2_t = cpool.tile([1, 768], f32r)
            nc.sync.dma_start(out=b2_t[:], in_=b2_d.ap())
            ones_t = cpool.tile([1, 128], f32r)
            nc.sync.dma_start(out=ones_t[:], in_=ones_d.ap())
            proj_t = cpool.tile([128, 256], f32)
            nc.sync.dma_start(out=proj_t[:], in_=proj_d.ap())
            segS = cpool.tile([128, NBk * 72], f32)
            shiftB = cpool.tile([128, 1], f32)
            nc.vector.memset(shiftB[:], -SHIFT_B)

            bpool = ctx.enter_context(tc.tile_pool(name="edges", bufs=3))
            tpool = ctx.enter_context(tc.tile_pool(name="work", bufs=2))
            hpool = ctx.enter_context(
                tc.tile_pool(name="psH", bufs=2, space="PSUM"))
            ppool = ctx.enter_context(
                tc.tile_pool(name="psA", bufs=2, space="PSUM"))
            spool = ctx.enter_context(
                tc.tile_pool(name="psS", bufs=2, space="PSUM"))

            for b in range(NBk):
                # ---- per-bucket bulk loads
                basis_b = bpool.tile([128, KB * 64], fp16, tag="basisb")
                nc.sync.dma_start(
                    out=basis_b[:],
                    in_=vap(basis_d.ap(), b * 128 * KB * 64,
                            [[KB * 64, 128], [1, KB * 64]]))
                basis_ba = basis_b[:]
                fu_b = bpool.tile([128, KB * 32], fp16, tag="fub")
                nc.sync.dma_start(
                    out=fu_b[:],
                    in_=vap(fu_d.ap(), b * 128 * KB * 32,
                            [[KB * 32, 128], [1, KB * 32]]))
                fu_ba = fu_b[:]
                eft_b = bpool.tile([32, KB * 128], f32r, tag="eftb")
                nc.sync.dma_start(
                    out=eft_b[:],
                    in_=vap(eft_d.ap(), b * KB * 128,
                            [[E_PAD, 32], [1, KB * 128]]))
                oh_b = bpool.tile([128, KB * 128], bf16, tag="ohb")
                nc.sync.dma_start(
                    out=oh_b[:],
                    in_=vap(oh_d.ap(), b * 128 * KB * 128,
                            [[KB * 128, 128], [1, KB * 128]]))

                seg = spool.tile([128, 72], f32, tag="seg")
                for g0, ng in GRPS:
                    # ---- MLP layer 1 for a group of chunks (f32r, PE)
                    hps = hpool.tile([64, 512], f32, tag="hps")
                    nc.tensor.matmul(out=hps[:, 0:ng * 128],
                                     lhsT=w1t_t[:],
                                     rhs=eft_b[:, g0 * 128:(g0 + ng) * 128],
                                     start=True, stop=True)
                    h64 = tpool.tile([64, 512], f32r, tag="h64")
                    nc.scalar.activation(h64[:, 0:ng * 128],
                                         hps[:, 0:ng * 128], Act.Relu,
                                         bias=b1_t[:, 0:1])

                    for i4 in range(ng):
                        i = g0 + i4
                        # ---- MLP layer 2 (f32r, PE) -> rw in PSUM
                        # bias b2 added via a rank-1 accumulating matmul
                        rw = ppool.tile([128, 768], f32, tag="rw")
                        hsl = h64[:, i4 * 128:(i4 + 1) * 128]
                        nc.tensor.matmul(out=rw[:, 0:512], lhsT=hsl,
                                         rhs=w2b_t[:, 0:512], start=True,
                                         stop=cfg.b2zero)
                        if not cfg.b2zero:
                            nc.tensor.matmul(out=rw[:, 0:512],
                                             lhsT=ones_t[:],
                                             rhs=b2_t[:, 0:512], start=False,
                                             stop=True)
                        nc.tensor.matmul(out=rw[:, 512:768], lhsT=hsl,
                                         rhs=w2b_t[:, 512:768], start=True,
                                         stop=cfg.b2zero)
                        if not cfg.b2zero:
                            nc.tensor.matmul(out=rw[:, 512:768],
                                             lhsT=ones_t[:],
                                             rhs=b2_t[:, 512:768],
                                             start=False, stop=True)
                        rw_s = tpool.tile([128, 768], fp16, tag="rws")
                        nc.scalar.activation(rw_s[:], rw[:], Act.Copy)

                        # ---- tmp2 = fU (x) basis, fp16
                        # pt[(m,k,d)] = fU[m,d] * basisT[k,d]
                        pt = tpool.tile([128, 512], fp16, tag="pt")
                        nc.gpsimd.tensor_tensor(
                            pt[:],
                            vap(fu_ba, i * 32,
                                [[KB * 32, 128], [4, 8], [0, 16], [1, 4]]),
                            vap(basis_ba, i * 64,
                                [[KB * 64, 128], [0, 8], [4, 16], [1, 4]]),
                            Alu.mult)
                        t1 = tpool.tile([128, 256], fp16, tag="t1")
                        nc.gpsimd.tensor_tensor(
                            t1[:],
                            vap(pt[:], 0, [[512, 128], [4, 128], [1, 2]]),
                            vap(pt[:], 2, [[512, 128], [4, 128], [1, 2]]),
                            Alu.add)
                        # tmp2[(d2,j=m*4+kk)] = t1[m,kk*4+d2,0]+t1[...,1]
                        tmp2 = tpool.tile([128, 128], fp16, tag="tmp")
                        tmp2a = tmp2[:]
                        nc.gpsimd.tensor_tensor(
                            vap(tmp2a, 0,
                                [[128, 128], [32, 4], [4, 8], [1, 4]]),
                            vap(t1[:], 0,
                                [[256, 128], [2, 4], [32, 8], [8, 4]]),
                            vap(t1[:], 1,
                                [[256, 128], [2, 4], [32, 8], [8, 4]]),
                            Alu.add)

                        # ---- conv = rw @ tmp2 : products + halving tree
                        pc = tpool.tile([128, 3072], fp16, tag="pc")
                        nc.vector.tensor_tensor(
                            pc[:],
                            vap(rw_s[:], 0,
                                [[768, 128], [32, 24], [0, 4], [1, 32]]),
                            vap(tmp2a, 0,
                                [[128, 128], [0, 24], [32, 4], [1, 32]]),
                            Alu.mult)
                        cv96 = tpool.tile([128, 96], fp16, tag="cv96")
                        cur, wj, lvl = pc[:], 32, 0
                        while wj > 2:
                            wj2 = wj // 2
                            pin = 96 * wj
                            nxt = tpool.tile([128, 96 * wj2], fp16,
                                             tag=f"ct{lvl}")
                            nc.vector.tensor_tensor(
                                nxt[:],
                                vap(cur, 0, [[pin, 128], [4 * wj, 24],
                                             [wj, 4], [1, wj2]]),
                                vap(cur, wj2, [[pin, 128], [4 * wj, 24],
                                               [wj, 4], [1, wj2]]),
                                Alu.add)
                            cur, wj, lvl = nxt[:], wj2, lvl + 1
                        nc.vector.tensor_tensor(
                            cv96[:],
                            vap(cur, 0, [[192, 128], [8, 24], [2, 4]]),
                            vap(cur, 1, [[192, 128], [8, 24], [2, 4]]),
                            Alu.add)
                        cva = cv96[:]

                        # ---- scores (fp16 products, f32 reduce)
                        ps = tpool.tile([128, 32], fp16, tag="ps")
                        nc.vector.tensor_tensor(
                            ps[:],
                            vap(cva, 0, [[96, 128], [1, 32]]),
                            vap(cva, 32, [[96, 128], [1, 32]]),
                            Alu.mult)
                        sc4 = tpool.tile([128, 4], f32, tag="sc4")
                        nc.vector.tensor_reduce(
                            sc4[:],
                            vap(ps[:], 0, [[32, 128], [8, 4], [1, 8]]),
                            mybir.AxisListType.X, Alu.add)
                        # scl = LeakyReLU(s * SCALE)
                        # (HW Lrelu activation ignores alpha)
                        scl0 = tpool.tile([128, 4], f32, tag="scl0")
                        nc.vector.tensor_scalar(scl0[:], sc4[:], SCALE, None,
                                                Alu.mult)
                        scl = tpool.tile([128, 4], f32, tag="scl")
                        nc.vector.scalar_tensor_tensor(
                            scl[:], scl0[:], 0.2, scl0[:], Alu.mult, Alu.max)

                        # ---- dual exp + payload (bf16 for range)
                        Y = tpool.tile([128, 72], bf16, tag="Y")
                        Ya = Y[:]
                        nc.scalar.activation(Y[:, 32:36], scl[:], Act.Exp)
                        nc.vector.tensor_scalar(
                            Y[:, 32:36], Y[:, 32:36], CLAMP_A, None, Alu.min)
                        nc.scalar.activation(Y[:, 68:72], scl[:], Act.Exp,
                                             bias=shiftB[:, 0:1])
                        eABx = tpool.tile([128, 64], bf16, tag="eABx")
                        nc.scalar.activation(
                            eABx[:],
                            vap(Ya, 32, [[72, 128], [36, 2], [1, 4], [0, 8]]),
                            Act.Copy)
                        nc.vector.tensor_tensor(
                            vap(Ya, 0, [[72, 128], [1, 32]]),
                            vap(cva, 64, [[96, 128], [1, 32]]),
                            eABx[:, 0:32], Alu.mult)
                        nc.vector.tensor_tensor(
                            vap(Ya, 36, [[72, 128], [1, 32]]),
                            vap(cva, 64, [[96, 128], [1, 32]]),
                            eABx[:, 32:64], Alu.mult)

                        # ---- one-hot segment matmul (bf16)
                        nc.tensor.matmul(
                            out=seg[:],
                            lhsT=vap(oh_b[:], i * 128,
                                     [[KB * 128, 128], [1, 128]]),
                            rhs=Y[:],
                            start=(i == 0), stop=(i == KB - 1))

                nc.scalar.activation(segS[:, b * 72:(b + 1) * 72], seg[:],
                                     Act.Copy)

            # ======== Phase 3: select pass, divide, project, store ========
            segA = segS[:]
            rdA = cpool.tile([128, NBk * 4], f32)
            nc.vector.tensor_scalar(
                rdA[:], vap(segA, 32, [[NBk * 72, 128], [72, NBk], [1, 4]]),
                1e-30, None, Alu.add)
            nc.vector.reciprocal(rdA[:], rdA[:])
            rdB = cpool.tile([128, NBk * 4], f32)
            nc.vector.tensor_scalar(
                rdB[:], vap(segA, 68, [[NBk * 72, 128], [72, NBk], [1, 4]]),
                1e-30, None, Alu.add)
            nc.vector.reciprocal(rdB[:], rdB[:])
            # selection mask per (node, head): 1.0 if den_A < SEL_TH
            msk = cpool.tile([128, NBk * 4], f32)
            nc.vector.tensor_scalar(
                msk[:], vap(segA, 32, [[NBk * 72, 128], [72, NBk], [1, 4]]),
                SEL_TH, None, Alu.is_lt)
            oA = cpool.tile([128, NBk * 32], f32)
            nc.vector.tensor_tensor(
                vap(oA[:], 0, [[NBk * 32, 128], [32, NBk], [8, 4], [1, 8]]),
                vap(segA, 0, [[NBk * 72, 128], [72, NBk], [8, 4], [1, 8]]),
                vap(rdA[:], 0, [[NBk * 4, 128], [4, NBk], [1, 4], [0, 8]]),
                Alu.mult)
            oB = cpool.tile([128, NBk * 32], f32)
            nc.vector.tensor_tensor(
                vap(oB[:], 0, [[NBk * 32, 128], [32, NBk], [8, 4], [1, 8]]),
                vap(segA, 36, [[NBk * 72, 128], [72, NBk], [8, 4], [1, 8]]),
                vap(rdB[:], 0, [[NBk * 4, 128], [4, NBk], [1, 4], [0, 8]]),
                Alu.mult)
            # blend: osc = oB + msk * (oA - oB)
            osc = cpool.tile([128, NBk * 32], f32)
            osca = osc[:]
            nc.vector.tensor_tensor(oA[:], oA[:], oB[:], Alu.subtract)
            nc.vector.tensor_tensor(
                vap(oA[:], 0, [[NBk * 32, 128], [32, NBk], [8, 4], [1, 8]]),
                vap(oA[:], 0, [[NBk * 32, 128], [32, NBk], [8, 4], [1, 8]]),
                vap(msk[:], 0, [[NBk * 4, 128], [4, NBk], [1, 4], [0, 8]]),
                Alu.mult)
            nc.vector.tensor_tensor(osc[:], oA[:], oB[:], Alu.add)
            res = cpool.tile([128, NBk * 32], f32)
            resa = res[:]
            scr = cpool.tile([128, NBk * 32], f32)
            scra = scr[:]
            for mp in range(8):
                tgt = resa if mp == 0 else scra
                nc.vector.tensor_tensor(
                    vap(tgt, 0, [[NBk * 32, 128], [32, NBk], [4, 8], [1, 4]]),
                    vap(osca, mp * 4,
                        [[NBk * 32, 128], [32, NBk], [0, 8], [1, 4]]),
                    vap(proj_t[:], mp * 32,
                        [[256, 128], [0, NBk], [4, 8], [1, 4]]),
                    Alu.mult)
                if mp > 0:
                    nc.vector.tensor_tensor(resa, resa, scra, Alu.add)
            nc.sync.dma_start(
                out=vap(out_d.ap(), 0, [[32, 128], [4096, NBk], [1, 32]]),
                in_=res[:])
    return nc


# ---------------------------------------------------------------------------
# Host-side prep
# ---------------------------------------------------------------------------
def _host_ln(features, ln_w, ln_b):
    f32 = np.float32
    feats = features.reshape(-1, M1, D1).astype(f32)
    onehot = np.eye(2, dtype=f32)[IX1]
    norms = np.sqrt((feats ** 2) @ onehot)
    x = norms.reshape(-1, 2, 8)
    mu = x.mean(-1, keepdims=True, dtype=f32).astype(f32)
    var = ((x - mu) ** 2).mean(-1, keepdims=True, dtype=f32).astype(f32)
    ln = (x - mu) / np.sqrt(var + LN_EPS) * ln_w + ln_b
    ln = np.maximum(ln, 0).astype(f32).reshape(-1, M1, 2)
    return (feats * (ln / (norms + EQ_EPS))[:, :, IX1]).astype(f32)


def _prep(inputs, cfg: Cfg = None):
    import ml_dtypes
    bfnp = ml_dtypes.bfloat16
    src = np.asarray(inputs["src"]).astype(np.int64)
    dst = np.asarray(inputs["dst"]).astype(np.int64)
    n_nodes = np.asarray(inputs["features"]).shape[0]
    # basis stored k-major per edge: (E, k=16, d=4)
    basis = np.asarray(inputs["basis"], np.float32).transpose(0, 2, 1)
    basis = np.ascontiguousarray(basis).reshape(-1, 64)
    ef = np.asarray(inputs["edge_feats"], np.float32)

    nb_l = cfg.nb if cfg is not None else NB
    nb_g = N_CORES * nb_l
    nodes_pad = nb_g * BUCKET_N

    deg = np.bincount(dst, minlength=nodes_pad)
    order = np.argsort(-deg, kind="stable")
    assign = np.empty(nodes_pad, dtype=np.int64)
    pos = np.empty(nodes_pad, dtype=np.int64)
    for r in range(BUCKET_N):
        sl = order[r * nb_g:(r + 1) * nb_g]
        buckets = np.arange(nb_g) if r % 2 == 0 else np.arange(nb_g)[::-1]
        assign[sl] = buckets
        pos[sl] = r
    loads = np.zeros(nb_g, dtype=np.int64)
    np.add.at(loads, assign[dst], 1)
    kb = int(math.ceil(loads.max() / 128.0))
    b2z = not np.any(np.asarray(inputs["b2"], np.float32))
    if cfg is None:
        cfg = Cfg(nb=nb_l, kb=kb, b2zero=b2z)
    assert kb <= cfg.kb, f"kb={kb} exceeds cfg.kb={cfg.kb}"

    # host LN + gather
    f = _host_ln(np.asarray(inputs["features"], np.float32),
                 np.asarray(inputs["ln_w"], np.float32),
                 np.asarray(inputs["ln_b"], np.float32))
    fU_all = f[src].reshape(-1, 32)

    eb = assign[dst]
    eorder = np.argsort(eb, kind="stable")
    bstart = np.searchsorted(eb[eorder], np.arange(nb_g + 1))

    E_PAD, CH, KB = cfg.e_pad, cfg.ch, cfg.kb
    slot_ar = np.arange(128, dtype=np.int64)
    in_maps = []
    for core in range(N_CORES):
        basis_s = np.zeros((E_PAD, 64), np.float16)
        fu_s = np.zeros((E_PAD, 32), np.float16)
        eft_s = np.zeros((32, E_PAD), np.float32)
        dstrel_s = np.full((E_PAD,), -1, np.int64)
        for lb in range(cfg.nb):
            gb = core * cfg.nb + lb
            eidx = eorder[bstart[gb]:bstart[gb + 1]]
            n = len(eidx)
            assert n <= KB * 128
            o = lb * KB * 128
            basis_s[o:o + n] = basis[eidx]
            fu_s[o:o + n] = fU_all[eidx]
            eft_s[:, o:o + n] = ef[eidx].T
            dstrel_s[o:o + n] = pos[dst[eidx]]
        # bucket-block layouts: (NB, 128, KB*w); edge (chunk i, part p)
        basis_bb = (basis_s.reshape(cfg.nb, KB, 128, 64)
                    .transpose(0, 2, 1, 3).reshape(cfg.nb, 128, KB * 64))
        fu_bb = (fu_s.reshape(cfg.nb, KB, 128, 32)
                 .transpose(0, 2, 1, 3).reshape(cfg.nb, 128, KB * 32))
        # host one-hot (bf16): (b, p, i*128 + slot)
        dr = dstrel_s.reshape(cfg.nb, KB, 128)
        oh = (dr[..., None] == slot_ar).astype(bfnp)
        oh_bb = oh.transpose(0, 2, 1, 3).reshape(cfg.nb, 128, KB * 128)
        in_maps.append({
            "basis_s": np.ascontiguousarray(basis_bb),
            "fu_s": np.ascontiguousarray(fu_bb),
            "eft_s": eft_s,
            "oh_s": np.ascontiguousarray(oh_bb),
        })

    w1 = np.asarray(inputs["w1"], np.float32)
    b1 = np.asarray(inputs["b1"], np.float32).reshape(64, 1)
    w2 = np.asarray(inputs["w2"], np.float32)
    b2 = np.asarray(inputs["b2"], np.float32)
    w2b = np.ascontiguousarray(w2.T).astype(np.float32)
    b2r = b2[None, :].astype(np.float32)
    projw = np.asarray(inputs["proj_w"], np.float32)
    ptbl_flat = np.zeros((256,), np.float32)
    for mpi in range(8):
        for m in range(8):
            for d in range(4):
                ptbl_flat[mpi * 32 + m * 4 + d] = projw[IX2[d] * 8 + m, mpi]
    ptbl = np.broadcast_to(ptbl_flat, (128, 256)).copy()
    ones_row = np.ones((1, 128), np.float32)
    for im in in_maps:
        im.update({
            "w1t_s": np.ascontiguousarray(w1.T),
            "b1_s": b1,
            "w2b_s": w2b,
            "b2r_s": b2r,
            "ones_s": ones_row,
            "proj_s": ptbl,
        })
    meta = {"assign": assign, "pos": pos, "n_nodes": n_nodes}
    return in_maps, meta, cfg


def _unshard(results, meta):
    out_cat = np.concatenate([r["out_s"] for r in results], axis=0)
    assign, pos, n = meta["assign"], meta["pos"], meta["n_nodes"]
    rows = assign[:n] * 128 + pos[:n]
    return out_cat[rows].reshape(n, M2, D2)


def _run(inputs, trace=False):
    _apply_patches()
    import concourse.bass as bass
    from concourse.bass_utils import run_bass_kernel_spmd

    in_maps, meta, cfg = _prep(inputs)
    nc = bass.Bass("TRN2", target_bir_lowering=False)
    build_kernel(nc, cfg)
    r = run_bass_kernel_spmd(nc, in_maps, core_ids=list(range(N_CORES)),
                             trace=trace)
    out = _unshard(r.results, meta)
    return out, r


def kernel(**inputs) -> np.ndarray:
    out, _ = _run(inputs, trace=False)
    return out.astype(np.float32)


# revision 13
# speedup vs baseline: 1.1229x; 1.1229x over previous
"""Trainium2 Bass kernel for nn_EquivariantTransformerBlock.

Strategy (8 NeuronCores, no collectives, no indirect DMA):
  - Host assigns each node to one of 320 "buckets" of 128 nodes (degree-
    balanced snake packing).  Core c owns buckets [40c, 40c+40); every edge
    goes to the core owning its dst bucket, so all segment sums are local.
  - Host computes the (tiny) equivariant LayerNorm and gathers f[src] so
    fU arrives as a dense per-edge input; all heavy compute (edge MLP,
    per-edge tensor contractions, attention, projection) runs on device.
  - Per-edge compute uses an edges-on-partitions layout (128 edges/chunk):
      * edge MLP on the TensorE in float32r (full PE rate, fp32-like
        precision),
      * the per-edge bilinear contractions (tmp = fU@basis, conv = rw@tmp)
        as broadcast-view products + halving-tree sums on the VectorE in
        FP16 (2x_1p DVE mode; fp16 keeps score error ~5e-4 relative, vs
        bf16 whose ~0.4% error is exp-amplified out of tolerance),
      * per-edge softmax scalars (LeakyReLU, exp, broadcasts, casts) on
        the ScalarE,
      * segment sums as one-hot matmuls (host-precomputed bf16 one-hot)
        accumulated in PSUM per bucket.
  - Softmax without per-node max: two exp variants per edge - A: clamped
    exp(s) (valid while den_A < 1e33), B: exp(s - 140) (valid for hot
    nodes) - and a per-node select after the segment sums.  Softmax is
    shift-invariant, so either variant matches the reference numerically.
"""

import math
from contextlib import ExitStack
from dataclasses import dataclass

import numpy as np

N_NODES = 40000
N_EDGES = 320000
M1, D1 = 8, 4
M2, D2 = 8, 4
LN_EPS = 1e-5
EQ_EPS = 1e-8
IX1 = np.array([0, 1, 1, 1])
IX2 = np.array([0, 1, 1, 1])

N_CORES = 8
BUCKET_N = 128
NB = 40
SCALE = 32.0 ** -0.5
SHIFT_B = 140.0      # pass-B exponent shift
CLAMP_A = 1e34       # pass-A exp clamp
SEL_TH = 1e33        # use pass B when den_A >= SEL_TH


@dataclass
class Cfg:
    nb: int
    kb: int
    b2zero: bool = False

    @property
    def ch(self):
        return self.nb * self.kb

    @property
    def e_pad(self):
        return self.ch * 128


# ---------------------------------------------------------------------------
# Patches: this walrus build allows at most ONE sync wait per instruction.
# ---------------------------------------------------------------------------
_PATCHED = False


def _apply_patches():
    global _PATCHED
    if _PATCHED:
        return
    _PATCHED = True
    import re as _re

    import orjson as _orjson

    import concourse.bass as _bass
    from concourse.tile import TileContext as _TC
    from concourse.vector_clock import ScopedClock as _SC, VectorClock as _VC

    def _drain_and_barrier(self, tick_clock, wait_clock):
        nc = self.nc
        gvals = [int(x) for x in _re.findall(r"\d+", repr(tick_clock.global_clock))]
        nz = [(p, v) for p, v in enumerate(gvals) if v > 0]
        if not nz:
            nc.sync.drain()
        for p, v in nz:
            pvc = _VC()
            pvc.require_at_least(p, v)
            d = nc.sync.drain()
            wait_clock.add_sem_waits(d.ins, _SC({None: pvc}))
        nc.all_engine_barrier()
        assert self.sems is not None
        popped = nc._tile_sem_poison_stack.pop()
        assert popped is self._sem_poison
        nc.clear_and_free_semaphores(list(self.sems.allocated().values()))
        nc.all_engine_barrier()

    def _split_multi_waits(data: bytes) -> bytes:
        j = _orjson.loads(data)
        for fn in j.get("functions", []):
            for bb in fn.get("blocks", []):
                out = []
                for ins in bb.get("instructions", []):
                    si = ins.get("sync_info")
                    ow = (si or {}).get("on_wait") or []
                    if len(ow) > 1:
                        for k, w in enumerate(ow[:-1]):
                            out.append({
                                "debug": ins.get("debug", 0),
                                "engine": ins["engine"],
                                "ins": [],
                                "name": f"{ins['name']}-spw{k}",
                                "opcode": "EventSemaphore",
                                "outs": [],
                                "sync_info": {"on_update": [], "on_wait": [w]},
                            })
                        si["on_wait"] = [ow[-1]]
                    out.append(ins)
                bb["instructions"] = out
        return _orjson.dumps(j)

    _orig_to_json_bytes = _bass.Bass.to_json_bytes

    def _to_json_bytes(self):
        return _split_multi_waits(_orig_to_json_bytes(self))

    _TC._drain_and_barrier = _drain_and_barrier
    _bass.Bass.to_json_bytes = _to_json_bytes


# ---------------------------------------------------------------------------
# Device kernel
# ---------------------------------------------------------------------------
def build_kernel(nc, cfg: Cfg):
    import concourse.bass as bass
    import concourse.mybir as mybir
    from concourse.tile import TileContext

    f32 = mybir.dt.float32
    f32r = mybir.dt.float32r
    fp16 = mybir.dt.float16
    bf16 = mybir.dt.bfloat16
    Alu = mybir.AluOpType
    Act = mybir.ActivationFunctionType

    NBk, KB, CH, E_PAD = cfg.nb, cfg.kb, cfg.ch, cfg.e_pad

    basis_d = nc.dram_tensor("basis_s", (NBk, 128, KB * 64), fp16,
                             kind="ExternalInput")
    fu_d = nc.dram_tensor("fu_s", (NBk, 128, KB * 32), fp16,
                          kind="ExternalInput")
    eft_d = nc.dram_tensor("eft_s", (32, E_PAD), f32r, kind="ExternalInput")
    oh_d = nc.dram_tensor("oh_s", (NBk, 128, KB * 128), bf16,
                          kind="ExternalInput")
    w1t_d = nc.dram_tensor("w1t_s", (32, 64), f32r, kind="ExternalInput")
    b1_d = nc.dram_tensor("b1_s", (64, 1), f32, kind="ExternalInput")
    w2b_d = nc.dram_tensor("w2b_s", (64, 768), f32r, kind="ExternalInput")
    b2_d = nc.dram_tensor("b2r_s", (1, 768), f32r, kind="ExternalInput")
    ones_d = nc.dram_tensor("ones_s", (1, 128), f32r, kind="ExternalInput")
    proj_d = nc.dram_tensor("proj_s", (128, 256), f32, kind="ExternalInput")
    out_d = nc.dram_tensor("out_s", (NBk * 128, 32), f32,
                           kind="ExternalOutput")

    def vap(base, offset, dims):
        return bass.AP(base.tensor, base.offset + offset, dims)

    # MLP1 group sizes (chunks per hps tile; 512 cols keeps f32r at full rate)
    GRPS = []
    i0 = 0
    while i0 < KB:
        ng = min(4, KB - i0)
        GRPS.append((i0, ng))
        i0 += ng

    with TileContext(nc) as tc:
        with ExitStack() as ctx:
            cpool = ctx.enter_context(tc.tile_pool(name="consts", bufs=1))
            w1t_t = cpool.tile([32, 64], f32r)
            nc.sync.dma_start(out=w1t_t[:], in_=w1t_d.ap())
            b1_t = cpool.tile([64, 1], f32)
            nc.sync.dma_start(out=b1_t[:], in_=b1_d.ap())
            w2b_t = cpool.tile([64, 768], f32r)
            nc.sync.dma_start(out=w2b_t[:], in_=w2b_d.ap())
            b2_t = cpool.tile([1, 768], f32r)
            nc.sync.dma_start(out=b2_t[:], in_=b2_d.ap())
            ones_t = cpool.tile([1, 128], f32r)
            nc.sync.dma_start(out=ones_t[:], in_=ones_d.ap())
            proj_t = cpool.tile([128, 256], f32)
            nc.sync.dma_start(out=proj_t[:], in_=proj_d.ap())
            segS = cpool.tile([128, NBk * 72], f32)
            shiftB = cpool.tile([128, 1], f32)
            nc.vector.memset(shiftB[:], -SHIFT_B)

            bpool = ctx.enter_context(tc.tile_pool(name="edges", bufs=3))
            tpool = ctx.enter_context(tc.tile_pool(name="work", bufs=2))
            hpool = ctx.enter_context(
                tc.tile_pool(name="psH", bufs=2, space="PSUM"))
            ppool = ctx.enter_context(
                tc.tile_pool(name="psA", bufs=2, space="PSUM"))
            spool = ctx.enter_context(
                tc.tile_pool(name="psS", bufs=2, space="PSUM"))

            for b in range(NBk):
                # ---- per-bucket bulk loads
                basis_b = bpool.tile([128, KB * 64], fp16, tag="basisb")
                nc.sync.dma_start(
                    out=basis_b[:],
                    in_=vap(basis_d.ap(), b * 128 * KB * 64,
                            [[KB * 64, 128], [1, KB * 64]]))
                basis_ba = basis_b[:]
                fu_b = bpool.tile([128, KB * 32], fp16, tag="fub")
                nc.sync.dma_start(
                    out=fu_b[:],
                    in_=vap(fu_d.ap(), b * 128 * KB * 32,
                            [[KB * 32, 128], [1, KB * 32]]))
                fu_ba = fu_b[:]
                eft_b = bpool.tile([32, KB * 128], f32r, tag="eftb")
                nc.sync.dma_start(
                    out=eft_b[:],
                    in_=vap(eft_d.ap(), b * KB * 128,
                            [[E_PAD, 32], [1, KB * 128]]))
                oh_b = bpool.tile([128, KB * 128], bf16, tag="ohb")
                nc.sync.dma_start(
                    out=oh_b[:],
                    in_=vap(oh_d.ap(), b * 128 * KB * 128,
                            [[KB * 128, 128], [1, KB * 128]]))

                seg = spool.tile([128, 72], f32, tag="seg")
                for g0, ng in GRPS:
                    # ---- MLP layer 1 for a group of chunks (f32r, PE)
                    hps = hpool.tile([64, 512], f32, tag="hps")
                    nc.tensor.matmul(out=hps[:, 0:ng * 128],
                                     lhsT=w1t_t[:],
                                     rhs=eft_b[:, g0 * 128:(g0 + ng) * 128],
                                     start=True, stop=True)
                    h64 = tpool.tile([64, 512], f32r, tag="h64")
                    nc.scalar.activation(h64[:, 0:ng * 128],
                                         hps[:, 0:ng * 128], Act.Relu,
                                         bias=b1_t[:, 0:1])

                    for i4 in range(ng):
                        i = g0 + i4
                        # ---- MLP layer 2 (f32r, PE) -> rw in PSUM
                        # bias b2 added via a rank-1 accumulating matmul
                        rw = ppool.tile([128, 768], f32, tag="rw")
                        hsl = h64[:, i4 * 128:(i4 + 1) * 128]
                        nc.tensor.matmul(out=rw[:, 0:512], lhsT=hsl,
                                         rhs=w2b_t[:, 0:512], start=True,
                                         stop=cfg.b2zero)
                        if not cfg.b2zero:
                            nc.tensor.matmul(out=rw[:, 0:512],
                                             lhsT=ones_t[:],
                                             rhs=b2_t[:, 0:512], start=False,
                                             stop=True)
                        nc.tensor.matmul(out=rw[:, 512:768], lhsT=hsl,
                                         rhs=w2b_t[:, 512:768], start=True,
                                         stop=cfg.b2zero)
                        if not cfg.b2zero:
                            nc.tensor.matmul(out=rw[:, 512:768],
                                             lhsT=ones_t[:],
                                             rhs=b2_t[:, 512:768],
                                             start=False, stop=True)
                        rw_s = tpool.tile([128, 768], fp16, tag="rws")
                        nc.scalar.activation(rw_s[:], rw[:], Act.Copy)

                        # ---- tmp2 = fU (x) basis, fp16
                        # pt[(m,k,d)] = fU[m,d] * basisT[k,d]
                        pt = tpool.tile([128, 512], fp16, tag="pt")
                        nc.vector.tensor_tensor(
                            pt[:],
                            vap(fu_ba, i * 32,
                                [[KB * 32, 128], [4, 8], [0, 16], [1, 4]]),
                            vap(basis_ba, i * 64,
                                [[KB * 64, 128], [0, 8], [4, 16], [1, 4]]),
                            Alu.mult)
                        t1 = tpool.tile([128, 256], fp16, tag="t1")
                        nc.vector.tensor_tensor(
                            t1[:],
                            vap(pt[:], 0, [[512, 128], [4, 128], [1, 2]]),
                            vap(pt[:], 2, [[512, 128], [4, 128], [1, 2]]),
                            Alu.add)
                        # tmp2[(d2,j=m*4+kk)] = t1[m,kk*4+d2,0]+t1[...,1]
                        tmp2 = tpool.tile([128, 128], fp16, tag="tmp")
                        tmp2a = tmp2[:]
                        nc.vector.tensor_tensor(
                            vap(tmp2a, 0,
                                [[128, 128], [32, 4], [4, 8], [1, 4]]),
                            vap(t1[:], 0,
                                [[256, 128], [2, 4], [32, 8], [8, 4]]),
                            vap(t1[:], 1,
                                [[256, 128], [2, 4], [32, 8], [8, 4]]),
                            Alu.add)

                        # ---- conv = rw @ tmp2 : products + halving tree
                        pc = tpool.tile([128, 3072], fp16, tag="pc")
                        nc.vector.tensor_tensor(
                            pc[:],
                            vap(rw_s[:], 0,
                                [[768, 128], [32, 24], [0, 4], [1, 32]]),
                            vap(tmp2a, 0,
                                [[128, 128], [0, 24], [32, 4], [1, 32]]),
                            Alu.mult)
                        cv96 = tpool.tile([128, 96], fp16, tag="cv96")
                        cur, wj, lvl = pc[:], 32, 0
                        while wj > 2:
                            wj2 = wj // 2
                            pin = 96 * wj
                            nxt = tpool.tile([128, 96 * wj2], fp16,
                                             tag=f"ct{lvl}")
                            nc.vector.tensor_tensor(
                                nxt[:],
                                vap(cur, 0, [[pin, 128], [4 * wj, 24],
                                             [wj, 4], [1, wj2]]),
                                vap(cur, wj2, [[pin, 128], [4 * wj, 24],
                                               [wj, 4], [1, wj2]]),
                                Alu.add)
                            cur, wj, lvl = nxt[:], wj2, lvl + 1
                        nc.vector.tensor_tensor(
                            cv96[:],
                            vap(cur, 0, [[192, 128], [8, 24], [2, 4]]),
                            vap(cur, 1, [[192, 128], [8, 24], [2, 4]]),
                            Alu.add)
                        cva = cv96[:]

                        # ---- scores (fp16 products, f32 reduce)
                        ps = tpool.tile([128, 32], fp16, tag="ps")
                        nc.vector.tensor_tensor(
                            ps[:],
                            vap(cva, 0, [[96, 128], [1, 32]]),
                            vap(cva, 32, [[96, 128], [1, 32]]),
                            Alu.mult)
                        sc4 = tpool.tile([128, 4], f32, tag="sc4")
                        nc.vector.tensor_reduce(
                            sc4[:],
                            vap(ps[:], 0, [[32, 128], [8, 4], [1, 8]]),
                            mybir.AxisListType.X, Alu.add)
                        # scl = LeakyReLU(s * SCALE)
                        # (HW Lrelu activation ignores alpha)
                        scl0 = tpool.tile([128, 4], f32, tag="scl0")
                        nc.vector.tensor_scalar(scl0[:], sc4[:], SCALE, None,
                                                Alu.mult)
                        scl = tpool.tile([128, 4], f32, tag="scl")
                        nc.vector.scalar_tensor_tensor(
                            scl[:], scl0[:], 0.2, scl0[:], Alu.mult, Alu.max)

                        # ---- dual exp + payload (bf16 for range)
                        Y = tpool.tile([128, 72], bf16, tag="Y")
                        Ya = Y[:]
                        nc.scalar.activation(Y[:, 32:36], scl[:], Act.Exp)
                        nc.vector.tensor_scalar(
                            Y[:, 32:36], Y[:, 32:36], CLAMP_A, None, Alu.min)
                        nc.scalar.activation(Y[:, 68:72], scl[:], Act.Exp,
                                             bias=shiftB[:, 0:1])
                        eABx = tpool.tile([128, 64], bf16, tag="eABx")
                        nc.scalar.activation(
                            eABx[:],
                            vap(Ya, 32, [[72, 128], [36, 2], [1, 4], [0, 8]]),
                            Act.Copy)
                        nc.vector.tensor_tensor(
                            vap(Ya, 0, [[72, 128], [1, 32]]),
                            vap(cva, 64, [[96, 128], [1, 32]]),
                            eABx[:, 0:32], Alu.mult)
                        nc.vector.tensor_tensor(
                            vap(Ya, 36, [[72, 128], [1, 32]]),
                            vap(cva, 64, [[96, 128], [1, 32]]),
                            eABx[:, 32:64], Alu.mult)

                        # ---- one-hot segment matmul (bf16)
                        nc.tensor.matmul(
                            out=seg[:],
                            lhsT=vap(oh_b[:], i * 128,
                                     [[KB * 128, 128], [1, 128]]),
                            rhs=Y[:],
                            start=(i == 0), stop=(i == KB - 1))

                nc.scalar.activation(segS[:, b * 72:(b + 1) * 72], seg[:],
                                     Act.Copy)

            # ======== Phase 3: select pass, divide, project, store ========
            segA = segS[:]
            rdA = cpool.tile([128, NBk * 4], f32)
            nc.vector.tensor_scalar(
                rdA[:], vap(segA, 32, [[NBk * 72, 128], [72, NBk], [1, 4]]),
                1e-30, None, Alu.add)
            nc.vector.reciprocal(rdA[:], rdA[:])
            rdB = cpool.tile([128, NBk * 4], f32)
            nc.vector.tensor_scalar(
                rdB[:], vap(segA, 68, [[NBk * 72, 128], [72, NBk], [1, 4]]),
                1e-30, None, Alu.add)
            nc.vector.reciprocal(rdB[:], rdB[:])
            # selection mask per (node, head): 1.0 if den_A < SEL_TH
            msk = cpool.tile([128, NBk * 4], f32)
            nc.vector.tensor_scalar(
                msk[:], vap(segA, 32, [[NBk * 72, 128], [72, NBk], [1, 4]]),
                SEL_TH, None, Alu.is_lt)
            oA = cpool.tile([128, NBk * 32], f32)
            nc.vector.tensor_tensor(
                vap(oA[:], 0, [[NBk * 32, 128], [32, NBk], [8, 4], [1, 8]]),
                vap(segA, 0, [[NBk * 72, 128], [72, NBk], [8, 4], [1, 8]]),
                vap(rdA[:], 0, [[NBk * 4, 128], [4, NBk], [1, 4], [0, 8]]),
                Alu.mult)
            oB = cpool.tile([128, NBk * 32], f32)
            nc.vector.tensor_tensor(
                vap(oB[:], 0, [[NBk * 32, 128], [32, NBk], [8, 4], [1, 8]]),
                vap(segA, 36, [[NBk * 72, 128], [72, NBk], [8, 4], [1, 8]]),
                vap(rdB[:], 0, [[NBk * 4, 128], [4, NBk], [1, 4], [0, 8]]),
                Alu.mult)
            # blend: osc = oB + msk * (oA - oB)
            osc = cpool.tile([128, NBk * 32], f32)
            osca = osc[:]
            nc.vector.tensor_tensor(oA[:], oA[:], oB[:], Alu.subtract)
            nc.vector.tensor_tensor(
                vap(oA[:], 0, [[NBk * 32, 128], [32, NBk], [8, 4], [1, 8]]),
                vap(oA[:], 0, [[NBk * 32, 128], [32, NBk], [8, 4], [1, 8]]),
                vap(msk[:], 0, [[NBk * 4, 128], [4, NBk], [1, 4], [0, 8]]),
                Alu.mult)
            nc.vector.tensor_tensor(osc[:], oA[:], oB[:], Alu.add)
            res = cpool.tile([128, NBk * 32], f32)
            resa = res[:]
            scr = cpool.tile([128, NBk * 32], f32)
            scra = scr[:]
            for mp in range(8):
                tgt = resa if mp == 0 else scra
                nc.vector.tensor_tensor(
                    vap(tgt, 0, [[NBk * 32, 128], [32, NBk], [4, 8], [1, 4]]),
                    vap(osca, mp * 4,
                        [[NBk * 32, 128], [32, NBk], [0, 8], [1, 4]]),
                    vap(proj_t[:], mp * 32,
                        [[256, 128], [0, NBk], [4, 8], [1, 4]]),
                    Alu.mult)
                if mp > 0:
                    nc.vector.tensor_tensor(resa, resa, scra, Alu.add)
            nc.sync.dma_start(
                out=vap(out_d.ap(), 0, [[32, 128], [4096, NBk], [1, 32]]),
                in_=res[:])
    return nc


# ---------------------------------------------------------------------------
# Host-side prep
# ---------------------------------------------------------------------------
def _host_ln(features, ln_w, ln_b):
    f32 = np.float32
    feats = features.reshape(-1, M1, D1).astype(f32)
    onehot = np.eye(2, dtype=f32)[IX1]
    norms = np.sqrt((feats ** 2) @ onehot)
    x = norms.reshape(-1, 2, 8)
    mu = x.mean(-1, keepdims=True, dtype=f32).astype(f32)
    var = ((x - mu) ** 2).mean(-1, keepdims=True, dtype=f32).astype(f32)
    ln = (x - mu) / np.sqrt(var + LN_EPS) * ln_w + ln_b
    ln = np.maximum(ln, 0).astype(f32).reshape(-1, M1, 2)
    return (feats * (ln / (norms + EQ_EPS))[:, :, IX1]).astype(f32)


def _prep(inputs, cfg: Cfg = None):
    import ml_dtypes
    bfnp = ml_dtypes.bfloat16
    src = np.asarray(inputs["src"]).astype(np.int64)
    dst = np.asarray(inputs["dst"]).astype(np.int64)
    n_nodes = np.asarray(inputs["features"]).shape[0]
    # basis stored k-major per edge: (E, k=16, d=4)
    basis = np.asarray(inputs["basis"], np.float32).transpose(0, 2, 1)
    basis = np.ascontiguousarray(basis).reshape(-1, 64)
    ef = np.asarray(inputs["edge_feats"], np.float32)

    nb_l = cfg.nb if cfg is not None else NB
    nb_g = N_CORES * nb_l
    nodes_pad = nb_g * BUCKET_N

    deg = np.bincount(dst, minlength=nodes_pad)
    order = np.argsort(-deg, kind="stable")
    assign = np.empty(nodes_pad, dtype=np.int64)
    pos = np.empty(nodes_pad, dtype=np.int64)
    for r in range(BUCKET_N):
        sl = order[r * nb_g:(r + 1) * nb_g]
        buckets = np.arange(nb_g) if r % 2 == 0 else np.arange(nb_g)[::-1]
        assign[sl] = buckets
        pos[sl] = r
    loads = np.zeros(nb_g, dtype=np.int64)
    np.add.at(loads, assign[dst], 1)
    kb = int(math.ceil(loads.max() / 128.0))
    b2z = not np.any(np.asarray(inputs["b2"], np.float32))
    if cfg is None:
        cfg = Cfg(nb=nb_l, kb=kb, b2zero=b2z)
    assert kb <= cfg.kb, f"kb={kb} exceeds cfg.kb={cfg.kb}"

    # host LN + gather
    f = _host_ln(np.asarray(inputs["features"], np.float32),
                 np.asarray(inputs["ln_w"], np.float32),
                 np.asarray(inputs["ln_b"], np.float32))
    fU_all = f[src].reshape(-1, 32)

    eb = assign[dst]
    eorder = np.argsort(eb, kind="stable")
    bstart = np.searchsorted(eb[eorder], np.arange(nb_g + 1))

    E_PAD, CH, KB = cfg.e_pad, cfg.ch, cfg.kb
    slot_ar = np.arange(128, dtype=np.int64)
    in_maps = []
    for core in range(N_CORES):
        basis_s = np.zeros((E_PAD, 64), np.float16)
        fu_s = np.zeros((E_PAD, 32), np.float16)
        eft_s = np.zeros((32, E_PAD), np.float32)
        dstrel_s = np.full((E_PAD,), -1, np.int64)
        for lb in range(cfg.nb):
            gb = core * cfg.nb + lb
            eidx = eorder[bstart[gb]:bstart[gb + 1]]
            n = len(eidx)
            assert n <= KB * 128
            o = lb * KB * 128
            basis_s[o:o + n] = basis[eidx]
            fu_s[o:o + n] = fU_all[eidx]
            eft_s[:, o:o + n] = ef[eidx].T
            dstrel_s[o:o + n] = pos[dst[eidx]]
        # bucket-block layouts: (NB, 128, KB*w); edge (chunk i, part p)
        basis_bb = (basis_s.reshape(cfg.nb, KB, 128, 64)
                    .transpose(0, 2, 1, 3).reshape(cfg.nb, 128, KB * 64))
        fu_bb = (fu_s.reshape(cfg.nb, KB, 128, 32)
                 .transpose(0, 2, 1, 3).reshape(cfg.nb, 128, KB * 32))
        # host one-hot (bf16): (b, p, i*128 + slot)
        dr = dstrel_s.reshape(cfg.nb, KB, 128)
        oh = (dr[..., None] == slot_ar).astype(bfnp)
        oh_bb = oh.transpose(0, 2, 1, 3).reshape(cfg.nb, 128, KB * 128)
        in_maps.append({
            "basis_s": np.ascontiguousarray(basis_bb),
            "fu_s": np.ascontiguousarray(fu_bb),
            "eft_s": eft_s,
            "oh_s": np.ascontiguousarray(oh_bb),
        })

    w1 = np.asarray(inputs["w1"], np.float32)
    b1 = np.asarray(inputs["b1"], np.float32).reshape(64, 1)
    w2 = np.asarray(inputs["w2"], np.float32)
    b2 = np.asarray(inputs["b2"], np.float32)
    w2b = np.ascontiguousarray(w2.T).astype(np.float32)
    b2r = b2[None, :].astype(np.float32)
    projw = np.asarray(inputs["proj_w"], np.float32)
    ptbl_flat = np.zeros((256,), np.float32)
    for mpi in range(8):
        for m in range(8):
            for d in range(4):
                ptbl_flat[mpi * 32 + m * 4 + d] = projw[IX2[d] * 8 + m, mpi]
    ptbl = np.broadcast_to(ptbl_flat, (128, 256)).copy()
    ones_row = np.ones((1, 128), np.float32)
    for im in in_maps:
        im.update({
            "w1t_s": np.ascontiguousarray(w1.T),
            "b1_s": b1,
            "w2b_s": w2b,
            "b2r_s": b2r,
            "ones_s": ones_row,
            "proj_s": ptbl,
        })
    meta = {"assign": assign, "pos": pos, "n_nodes": n_nodes}
    return in_maps, meta, cfg


def _unshard(results, meta):
    out_cat = np.concatenate([r["out_s"] for r in results], axis=0)
    assign, pos, n = meta["assign"], meta["pos"], meta["n_nodes"]
    rows = assign[:n] * 128 + pos[:n]
    return out_cat[rows].reshape(n, M2, D2)


def _run(inputs, trace=False):
    _apply_patches()
    import concourse.bass as bass
    from concourse.bass_utils import run_bass_kernel_spmd

    in_maps, meta, cfg = _prep(inputs)
    nc = bass.Bass("TRN2", target_bir_lowering=False)
    build_kernel(nc, cfg)
    r = run_bass_kernel_spmd(nc, in_maps, core_ids=list(range(N_CORES)),
                             trace=trace)
    out = _unshard(r.results, meta)
    return out, r


def kernel(**inputs) -> np.ndarray:
    out, _ = _run(inputs, trace=False)
    return out.astype(np.float32)
